# revision 29
# baseline (speedup 1.0000x reference)
"""MoE expert FFN (forward_all + top-2 routing combine) on 8 TRN2 NeuronCores.

Strategy: the routing tensor has exactly TOP_K=2 nonzeros per token, so only
the routed (token, expert) pairs contribute. Host dispatch gathers each
expert's routed tokens, pads to a fixed per-slot capacity, and runs
expert-parallel on 8 cores (2 expert slots per core). Each core computes
y^T = gate * (w2^T @ gelu(w1^T @ x^T + b1)) for its slots' tokens; the host
scatter-adds the per-expert outputs back into [N, DIM].

Slots are ASYMMETRIC: seed-0 expert counts range 911..1153, so slot 0 (cap
C0=1153) takes the 8 largest experts and slot 1 (cap C1=1026) the 8 smallest
(9th-largest count; c1+c9 is the provably minimal 2-slot capacity sum). Total
padded columns per core = 2179 with zero host fallback on seed-0 data; any
count shift on other inputs falls back to an exact host path for the
overflow tokens only, so correctness never depends on the capacities.

Matmuls run in float16 (f32 PSUM accumulate; bias+gelu in f32); gates and
y outputs are f16 as well (rel err ~5e-4 vs the f32 reference).

Schedule: k-outer over 8 PSUM banks so matmuls consume weight slices as DMA
delivers them. Nearly all DMAs ride the Sync HW queue in DEADLINE order:
chunk 0's x slices + w1 a-halves (cols 0..1024) stream first - stage 1's
first half-pass reads only those - then the w1 b-halves, then w2; this keeps
the HBM-bound 5MB startup window supply-matched (interleaving a/b halves
costs ~2us; cross-queue dep chains couple through the shared ~10-semaphore
DMA pool and serialize - measured much worse - so ordering is by emission
only). w1 is pre-split on the host into separate a/b DRAM tensors and
slot 0's chunk-0 x is duplicated in [ko, p, n] layout (x0c), so every
startup descriptor is one contiguous linear read instead of 1-2KB bursts. The
Scalar HW queue carries just expert 0's first x k-slice and b1 (so matmul 0
unblocks sooner) and the final chunk's y DMAs (the tail drain skips the
Sync ring). Seven dummy matmuls on a memset tile run during the fixed ~7us
engine preamble and bridge gaplessly into the first real matmul, so the PE
HAM clock gate (free-running 3.413us activity windows; flip lands 3.4-6.8us
after the busy span starts, phase luck) flips as early as possible - the
first one-to-two k-groups still run at cold 1.2 GHz, the main residual
variance. Every chunk is >=320 tokens so the per-matmul LDWEIGHTS (~104ns)
stays hidden under the moving-operand stream (FD/2.4GHz + 2.5ns per matmul;
a 257-token chunk measurably pays +3ns/matmul). Last chunk runs stage 2
m-outer so early m tiles' gate-mul + y DMAs overlap the remaining matmuls.
Measured 252.4-256.6us, median ~253.6 (phases within ~1% of the
matmul-stream model; residue = fixed preamble/postamble ~9.5us, DMA ramp,
HAM phase luck, and ~1.6ns/MM NX fine structure).
"""

import math
from contextlib import ExitStack

import numpy as np

import concourse.mybir as mybir
import concourse.tile as tile
from concourse import bacc
from concourse.bass_utils import run_bass_kernel_spmd

N, DIM, E, EXPERT_DIM = 8192, 1024, 16, 2048
N_CORES = 8
N_SLOTS = 2  # expert slots per core
P = 128

# Per-slot capacities and chunking (seed-0 counts: max 1153, 9th-largest 1026)
C0, C1 = 1153, 1026
CHUNKS0 = [512, 321, 320]
CHUNKS1 = [386, 320, 320]
assert sum(CHUNKS0) == C0 and sum(CHUNKS1) == C1

KO1 = DIM // P  # 8 contraction tiles, stage 1
MO1 = EXPERT_DIM // P  # 16 output tiles, stage 1
KO2 = EXPERT_DIM // P  # 16 contraction tiles, stage 2
MO2 = DIM // P  # 8 output tiles, stage 2

DUMMY_N, DUMMY_FD = 7, 512  # PE warm-up matmuls during the engine preamble

MM_DTYPE = "f16"
TRACE = False  # set by test.py to capture an NTFF profile
LAST_EXEC_NS = None
LAST_TRACE_PATH = None
ACT_FUNC = None  # default Gelu; sim_check overrides (CoreSim lacks Gelu)

_NC_CACHE = {}


def _build_nc():
    f32 = mybir.dt.float32
    mdt = mybir.dt.float16
    odt = mybir.dt.float16

    nc = bacc.Bacc("TRN2", target_bir_lowering=False, debug=False, num_devices=N_CORES)
    xt0 = nc.dram_tensor("xt0", [DIM, C0], mdt, kind="ExternalInput").ap()
    # chunk 0 of slot 0 duplicated in [ko, p, n] layout: each k-slice is one
    # contiguous 128KB read during the bandwidth-critical startup window
    x0c = nc.dram_tensor(
        "x0c", [KO1, P, CHUNKS0[0]], mdt, kind="ExternalInput"
    ).ap()
    xt1 = nc.dram_tensor("xt1", [DIM, C1], mdt, kind="ExternalInput").ap()
    HALF = EXPERT_DIM // 2
    w1a = nc.dram_tensor("w1a", [N_SLOTS, DIM, HALF], mdt, kind="ExternalInput").ap()
    w1b = nc.dram_tensor("w1b", [N_SLOTS, DIM, HALF], mdt, kind="ExternalInput").ap()
    b1 = nc.dram_tensor("b1", [N_SLOTS, P, MO1], f32, kind="ExternalInput").ap()
    w2 = nc.dram_tensor("w2", [N_SLOTS, EXPERT_DIM, DIM], mdt, kind="ExternalInput").ap()
    g0 = nc.dram_tensor("g0", [P, C0], mdt, kind="ExternalInput").ap()
    g1 = nc.dram_tensor("g1", [P, C1], mdt, kind="ExternalInput").ap()
    yt0 = nc.dram_tensor("yt0", [DIM, C0], odt, kind="ExternalOutput").ap()
    yt1 = nc.dram_tensor("yt1", [DIM, C1], odt, kind="ExternalOutput").ap()

    gelu = ACT_FUNC or mybir.ActivationFunctionType.Gelu
    GRP = 8  # psum tiles per interleaved matmul group (= PSUM banks)

    xts, gts, yts = [xt0, xt1], [g0, g1], [yt0, yt1]
    slot_chunks = [CHUNKS0, CHUNKS1]

    with tile.TileContext(nc) as tc, ExitStack() as ctx:
        w1_pool = ctx.enter_context(tc.tile_pool(name="w1", bufs=2))
        w2_pool = ctx.enter_context(tc.tile_pool(name="w2", bufs=1))
        b1_pool = ctx.enter_context(tc.tile_pool(name="b1", bufs=2))
        x_pool = ctx.enter_context(tc.tile_pool(name="x", bufs=4))
        g_pool = ctx.enter_context(tc.tile_pool(name="g", bufs=3))
        h_pool = ctx.enter_context(tc.tile_pool(name="h", bufs=2))
        y_pool = ctx.enter_context(tc.tile_pool(name="y", bufs=6))
        ps_pool = ctx.enter_context(tc.tile_pool(name="ps", bufs=GRP, space="PSUM"))
        dm_pool = ctx.enter_context(tc.tile_pool(name="dm", bufs=1))

        # PE warm-up: dep-free matmuls on an uninitialized tile run while the
        # startup DMAs are still in flight, flipping the HAM clock gate to
        # 2.4 GHz before the first real matmul. Their PSUM bank is reclaimed
        # by the pool (first real group overwrites with start=True).
        dummy = dm_pool.tile([P, DUMMY_FD], mdt, name="dummy")
        nc.gpsimd.memset(dummy[:], 1.0)
        dps = ps_pool.tile([P, DUMMY_FD], f32, tag="ps", name="dummy_ps")
        for i in range(DUMMY_N):
            nc.tensor.matmul(dps[:], dummy[:, :P], dummy[:], start=True, stop=True)

        w_gate = None  # last weight DMA of the previous expert slot
        for e in range(N_SLOTS):
            chunks = slot_chunks[e]
            offs = [sum(chunks[:i]) for i in range(len(chunks))]

            # ---- input DMAs, all on the Sync HW queue in supply order ----
            # Chunk 0's x k-slices interleave with w1's k-slices (halved so
            # the first matmul group's RAW dep clears at half the bytes): the
            # single queue naturally rations HBM between the two streams and
            # the ~0.6us per-descriptor issue cost paces supply just ahead of
            # chunk 0 stage 1's consumption.
            w1_t = w1_pool.tile([P, KO1, EXPERT_DIM], mdt, tag="w1", name=f"w1_{e}")
            w1ar = w1a[e].rearrange("(ko p) m -> p ko m", p=P)
            w1br = w1b[e].rearrange("(ko p) m -> p ko m", p=P)
            w2_t = w2_pool.tile([P, KO2, DIM], mdt, tag="w2", name=f"w2_{e}")
            w2r = w2[e].rearrange("(ko p) m -> p ko m", p=P)
            xr = xts[e].rearrange("(ko p) n -> p ko n", p=P)
            b1_t = b1_pool.tile([P, MO1], f32, name=f"b1_{e}")
            half = GRP * P
            x_ts, g_ts = [], []
            last_w1 = None
            for t, tok in enumerate(chunks):
                tsl = slice(offs[t], offs[t] + tok)
                x_t = x_pool.tile([P, KO1, tok], mdt, tag="x", name=f"x_{e}_{t}")
                if t == 0:
                    # deadline-order supply: stage 1's first half-pass reads
                    # only x + w1 cols 0..half, so those stream first; the b
                    # halves (needed one half-pass later) follow them
                    for ko in range(KO1):
                        if e == 0 and ko == 0:
                            dx = nc.scalar.dma_start(x_t[:, ko], x0c[ko])
                        elif e == 0:
                            dx = nc.sync.dma_start(x_t[:, ko], x0c[ko])
                        else:
                            dx = nc.sync.dma_start(x_t[:, ko], xr[:, ko, tsl])
                        da = nc.sync.dma_start(
                            w1_t[:, ko, :half], w1ar[:, ko]
                        )
                        if w_gate is not None:
                            for d in (dx, da):
                                tile.add_dep_helper(
                                    d.ins, w_gate, reason="weight phase order"
                                )
                    for ko in range(KO1):
                        db = nc.sync.dma_start(
                            w1_t[:, ko, half:], w1br[:, ko]
                        )
                        if w_gate is not None:
                            tile.add_dep_helper(
                                db.ins, w_gate, reason="weight phase order"
                            )
                        last_w1 = db
                    (nc.scalar if e == 0 else nc.sync).dma_start(b1_t[:], b1[e])
                    w2_segs = [(a, a + 1) for a in range(4)] + [
                        (a, a + 2) for a in range(4, KO2, 2)
                    ]
                    for a, b in w2_segs:
                        d = nc.sync.dma_start(w2_t[:, a:b], w2r[:, a:b])
                        tile.add_dep_helper(
                            d.ins, last_w1.ins, reason="w2 behind w1"
                        )
                    w_gate = d.ins
                else:
                    d = nc.sync.dma_start(x_t[:], xr[:, :, tsl])
                    tile.add_dep_helper(
                        d.ins, last_w1.ins, reason="x prefetch after w1"
                    )
                g_t = g_pool.tile([P, tok], mdt, tag="g", name=f"g_{e}_{t}")
                dg = nc.sync.dma_start(g_t[:], gts[e][:, tsl])
                tile.add_dep_helper(dg.ins, last_w1.ins, reason="g after w1")
                x_ts.append(x_t)
                g_ts.append(g_t)

            # ---- compute ----
            for t, tok in enumerate(chunks):
                tsl = slice(offs[t], offs[t] + tok)
                x_t, g_t = x_ts[t], g_ts[t]

                # stage 1: h^T = gelu(w1^T @ x^T + b1), k-interleaved over 8
                # PSUM banks so matmuls start as soon as each weight slice lands
                h_t = h_pool.tile([P, MO1, tok], mdt, tag="h", name=f"h_{e}_{t}")
                for hf in range(MO1 // GRP):
                    pss = [
                        ps_pool.tile(
                            [P, tok], mybir.dt.float32, tag="ps",
                            name=f"ps_{e}_{t}_{hf}_{i}",
                        )
                        for i in range(GRP)
                    ]
                    for ko in range(KO1):
                        for i in range(GRP):
                            mo = hf * GRP + i
                            nc.tensor.matmul(
                                pss[i][:],
                                w1_t[:, ko, mo * P : (mo + 1) * P],
                                x_t[:, ko],
                                start=(ko == 0),
                                stop=(ko == KO1 - 1),
                            )
                    for i in range(GRP):
                        mo = hf * GRP + i
                        nc.scalar.activation(
                            h_t[:, mo], pss[i][:], gelu, bias=b1_t[:, mo : mo + 1]
                        )

                # stage 2: y^T = gate * (w2^T @ h^T), k-interleaved in two
                # 4-bank groups - except the very last chunk, which runs
                # m-outer so early m tiles' gate-mul + output DMAs overlap the
                # remaining matmuls (shorter tail).
                last = e == N_SLOTS - 1 and t == len(chunks) - 1
                if last:
                    for mo in range(MO2):
                        ps2 = ps_pool.tile(
                            [P, tok], mybir.dt.float32, tag="ps",
                            name=f"ps2_{e}_{t}_{mo}",
                        )
                        for ko in range(KO2):
                            nc.tensor.matmul(
                                ps2[:],
                                w2_t[:, ko, mo * P : (mo + 1) * P],
                                h_t[:, ko],
                                start=(ko == 0),
                                stop=(ko == KO2 - 1),
                            )
                        y_t = y_pool.tile([P, tok], odt, tag="y", name=f"y_{e}_{t}_{mo}")
                        nc.vector.tensor_mul(y_t[:], ps2[:], g_t[:])
                        nc.scalar.dma_start(yts[e][mo * P : (mo + 1) * P, tsl], y_t[:])
                else:
                    G2 = MO2 // 2
                    for h2 in range(2):
                        pss2 = [
                            ps_pool.tile(
                                [P, tok], mybir.dt.float32, tag="ps",
                                name=f"ps2_{e}_{t}_{h2}_{i}",
                            )
                            for i in range(G2)
                        ]
                        for ko in range(KO2):
                            for i in range(G2):
                                mo = h2 * G2 + i
                                nc.tensor.matmul(
                                    pss2[i][:],
                                    w2_t[:, ko, mo * P : (mo + 1) * P],
                                    h_t[:, ko],
                                    start=(ko == 0),
                                    stop=(ko == KO2 - 1),
                                )
                        for i in range(G2):
                            mo = h2 * G2 + i
                            y_t = y_pool.tile(
                                [P, tok], odt, tag="y", name=f"y_{e}_{t}_{mo}"
                            )
                            nc.vector.tensor_mul(y_t[:], pss2[i][:], g_t[:])
                            nc.sync.dma_start(
                                yts[e][mo * P : (mo + 1) * P, tsl], y_t[:]
                            )

    nc.compile()
    return nc


def _get_nc():
    if "nc" not in _NC_CACHE:
        _NC_CACHE["nc"] = _build_nc()
    return _NC_CACHE["nc"]


def _install_ntff_hook():
    """Register the axon NTFF profile hook if the image's antenv lacks it."""
    import sys
    import types

    try:
        from antenv.axon_hooks import get_axon_ntff_profile_hook  # noqa: F401

        return True
    except ImportError:
        pass
    try:
        from trn_agent_boot.trn_boot import _ntff_profile_via_ctypes

        hook = _ntff_profile_via_ctypes("/opt/axon/libaxon_pjrt.so")
        if hook is None:
            return False
        mod = types.ModuleType("antenv.axon_hooks")
        state = {"hook": hook}
        mod.set_axon_ntff_profile_hook = lambda h: state.__setitem__("hook", h)
        mod.get_axon_ntff_profile_hook = lambda: state["hook"]
        sys.modules["antenv.axon_hooks"] = mod
        return True
    except Exception:
        return False


def _gelu_exact(v):
    # overflow fallback only; matches jax.nn.gelu(approximate=False)
    erf = np.vectorize(math.erf)
    return v * 0.5 * (1.0 + erf(v / math.sqrt(2.0)))


def kernel(x, routing_tensor, w1, b1, w2):
    global LAST_EXEC_NS, LAST_TRACE_PATH
    x = np.ascontiguousarray(np.asarray(x, np.float32))
    routing_tensor = np.asarray(routing_tensor, np.float32)
    w1 = np.asarray(w1, np.float32)
    b1 = np.asarray(b1, np.float32)
    w2 = np.asarray(w2, np.float32)

    # host dispatch: per-expert routed token lists; 8 largest experts fill
    # slot 0 (cap C0), 8 smallest fill slot 1 (cap C1)
    idx_list = [np.nonzero(routing_tensor[:, e])[0] for e in range(E)]
    counts = np.array([len(i) for i in idx_list])
    order = np.argsort(-counts, kind="stable")
    slot_experts = [order[:N_CORES], order[N_CORES:]]
    caps = [C0, C1]
    overflow = []  # (expert, token indices beyond cap) - empty for seed-0 data

    in_maps = []
    for c in range(N_CORES):
        m = {}
        es = []
        for s in range(N_SLOTS):
            e = int(slot_experts[s][c])
            es.append(e)
            idx = idx_list[e]
            if len(idx) > caps[s]:
                overflow.append((e, idx[caps[s] :]))
                idx = idx[: caps[s]]
                idx_list[e] = idx
            cnt = len(idx)
            xt = np.zeros((DIM, caps[s]), np.float16)
            xt[:, :cnt] = x[idx].T
            g = np.zeros((P, caps[s]), np.float16)
            g[:, :cnt] = routing_tensor[idx, e][None, :]
            m[f"xt{s}"] = xt
            if s == 0:
                m["x0c"] = np.ascontiguousarray(
                    xt[:, : CHUNKS0[0]].reshape(KO1, P, CHUNKS0[0])
                )
            m[f"g{s}"] = g
        w1c = w1[es].astype(np.float16)
        m["w1a"] = np.ascontiguousarray(w1c[:, :, : EXPERT_DIM // 2])
        m["w1b"] = np.ascontiguousarray(w1c[:, :, EXPERT_DIM // 2 :])
        m["b1"] = np.ascontiguousarray(
            b1[es].reshape(N_SLOTS, MO1, P).transpose(0, 2, 1)
        )
        m["w2"] = np.ascontiguousarray(w2[es], dtype=np.float16)
        in_maps.append(m)

    nc = _get_nc()
    core_ids = list(range(N_CORES))
    if TRACE and _install_ntff_hook():
        import concourse.bass_utils as _bu

        _bu.upload_artifacts = lambda tmpdir: tmpdir  # zero-egress container
        try:
            res = run_bass_kernel_spmd(nc, in_maps, core_ids, trace=True)
            LAST_EXEC_NS = res.exec_time_ns
            LAST_TRACE_PATH = (
                res.instructions_and_trace[1] if res.instructions_and_trace else None
            )
        except Exception:
            res = run_bass_kernel_spmd(nc, in_maps, core_ids)
    else:
        res = run_bass_kernel_spmd(nc, in_maps, core_ids)

    out = np.zeros((N, DIM), np.float32)
    for c in range(N_CORES):
        for s in range(N_SLOTS):
            e = int(slot_experts[s][c])
            idx = idx_list[e]
            yt = res.results[c][f"yt{s}"]  # [DIM, cap] f16
            out[idx] += yt[:, : len(idx)].T.astype(np.float32)

    for e, idx in overflow:
        h = _gelu_exact(x[idx] @ w1[e] + b1[e])
        out[idx] += (h @ w2[e]) * routing_tensor[idx, e][:, None]

    return out
